# revision 2
# baseline (speedup 1.0000x reference)
"""Causal attention (RMSNorm + QKV proj + causal softmax attention) on 8 TRN2
NeuronCores — bf16 matmul version.

Math (per reference):
    xn   = x / max(||x_row||, 1e-12) * sqrt(D) * gamma
    qkv  = xn @ w_qkv            -> q,k,v heads of dim 64
    q   *= D**-0.5
    out[b,h] = softmax_causal(q k^T) v

Sharding: 16 (batch, head) pairs over 8 cores -> core c gets batch c//4 and
heads {2*(c%4), 2*(c%4)+1}. Each core runs the same single-core Bass program
(SPMD) on its shard; gamma/weight slices are replicated per core.

Per-core kernel layout (all PE matmuls in bf16 — fp32 runs power-throttled
at ~half rate on TRN2; bf16 streams 1 row/cycle):
  - row scale s = sqrt(D)/||x_row||: square+accum on ScalarE, sqrt/recip on
    Scalar/DVE; x scaled to bf16 on GpSimd (otherwise-idle engine)
  - x transposed to d-major via PE transposes in bf16 (PSUM bf16 tiles)
  - projections: W (gamma folded, q also 1/32) bf16 stationary, xT moving ->
    qT/kT [128=2*64 feat, 4096] bf16; vT re-transposed and packed bf16 as
    [v_h0 | ones | v_h1 | ones] so the AV matmul also yields the softmax
    denominator as output row 64.
  - attention transposed: simT[j,i] = kT.T qT per 128-key x 512-query block,
    both heads' blocks land in one 2-bank PSUM tile [128,1024] so a single
    ScalarE exp instruction covers both; causal diagonal strip zeroed by a
    precomputed triangular bf16 mask multiply on DVE; AV accumulates in PSUM
    over key blocks. Final [65,512] blocks PE-transposed back to token-major,
    divided by the denominator row, DMA'd out.
  - ts (projection superblocks) and ib (attention superblocks) are emitted
    interleaved so every engine queue pipelines across the two phases.
"""

import numpy as np
from contextlib import ExitStack

import concourse.bass as bass
import concourse.tile as tile
from concourse import bacc, mybir
from concourse.masks import make_identity

F32 = mybir.dt.float32
BF16 = mybir.dt.bfloat16
AF = mybir.ActivationFunctionType
ALU = mybir.AluOpType

B, N, D = 2, 4096, 1024
HEADS, DH = 8, 64
NT = N // 128      # 32 token tiles
NSUP = N // 512    # 8 token superblocks
DC = D // 128      # 8 contraction chunks
VW = 2 * (DH + 1)  # 130: [v_h0 | ones | v_h1 | ones] per key tile


def build_program():
    nc = bacc.Bacc("TRN2", target_bir_lowering=False, debug=False)

    x = nc.dram_tensor("x", [N, D], F32, kind="ExternalInput").ap()
    wq = nc.dram_tensor("wq", [D, 128], F32, kind="ExternalInput").ap()
    wk = nc.dram_tensor("wk", [D, 128], F32, kind="ExternalInput").ap()
    wv = nc.dram_tensor("wv", [D, 128], F32, kind="ExternalInput").ap()
    gamma = nc.dram_tensor("gamma", [D], F32, kind="ExternalInput").ap()
    out = nc.dram_tensor("out", [2, N, DH], F32, kind="ExternalOutput").ap()

    with tile.TileContext(nc) as tc, ExitStack() as ctx:
        consts = ctx.enter_context(tc.tile_pool(name="consts", bufs=1))
        wpool = ctx.enter_context(tc.tile_pool(name="wpool", bufs=1))
        resid = ctx.enter_context(tc.tile_pool(name="resid", bufs=1))
        xpool = ctx.enter_context(tc.tile_pool(name="xpool", bufs=5))
        sqpool = ctx.enter_context(tc.tile_pool(name="sqpool", bufs=2))
        spool = ctx.enter_context(tc.tile_pool(name="spool", bufs=12))
        xtpool = ctx.enter_context(tc.tile_pool(name="xtpool", bufs=2))
        vtpool = ctx.enter_context(tc.tile_pool(name="vtpool", bufs=2))
        expool = ctx.enter_context(tc.tile_pool(name="expool", bufs=6))
        opool = ctx.enter_context(tc.tile_pool(name="opool", bufs=2))
        finpool = ctx.enter_context(tc.tile_pool(name="finpool", bufs=4))
        psSim = ctx.enter_context(tc.tile_pool(name="psSim", bufs=2, space="PSUM"))
        psAv = ctx.enter_context(tc.tile_pool(name="psAv", bufs=2, space="PSUM"))
        psWork = ctx.enter_context(tc.tile_pool(name="psWork", bufs=2, space="PSUM"))

        # ---- constants ---------------------------------------------------
        ident = consts.tile([128, 128], F32)
        make_identity(nc, ident[:])
        identb = consts.tile([128, 128], BF16)
        nc.vector.tensor_copy(identb[:], ident[:])

        # tri[p, h*128+f] = 1 if p <= f else 0 (causal keep-mask, bf16)
        tri = consts.tile([128, 256], BF16)
        nc.vector.memset(tri[:], 1.0)
        for h in range(2):
            nc.gpsimd.affine_select(
                out=tri[:, h * 128:(h + 1) * 128],
                in_=tri[:, h * 128:(h + 1) * 128],
                compare_op=ALU.is_ge, fill=0.0, base=0,
                channel_multiplier=-1, pattern=[[1, 128]],
            )

        # ---- weights: fold gamma (and D**-0.5 for q), cast bf16 ----------
        w_raw = {}
        w_sb = {}
        for name, w in (("q", wq), ("k", wk), ("v", wv)):
            t = wpool.tile([128, DC, 128], F32, tag=f"wr{name}")
            for c in range(DC):
                nc.sync.dma_start(t[:, c, :], w[c * 128:(c + 1) * 128, :])
            w_raw[name] = t
            w_sb[name] = wpool.tile([128, DC, 128], BF16, tag=f"w{name}", name=f"w_{name}")

        g_sb = wpool.tile([128, DC], F32, tag="g")
        nc.sync.dma_start(g_sb[:], gamma.rearrange("(c p) -> p c", p=128))
        gq_sb = wpool.tile([128, DC], F32, tag="gq")
        nc.scalar.mul(gq_sb[:], g_sb[:], float(D) ** -0.5)

        for name, gt in (("q", gq_sb), ("k", g_sb), ("v", g_sb)):
            for c in range(DC):
                nc.vector.tensor_scalar_mul(
                    w_sb[name][:, c, :], w_raw[name][:, c, :], gt[:, c:c + 1])

        # ---- residents ---------------------------------------------------
        qT = resid.tile([128, N], BF16, tag="qT")
        kT = resid.tile([128, N], BF16, tag="kT")
        # v_ext cols per key tile jj: [v_h0(64) | 1 | v_h1(64) | 1]
        v_ext = resid.tile([128, NT * VW], BF16, tag="vext")
        nc.vector.memset(v_ext[:], 1.0)

        # ---- phase emitters ---------------------------------------------
        def emit_ts(ts):
            """Norm + transpose + projections for tokens [ts*512, (ts+1)*512)."""
            xbs = []
            for tt in range(4):
                t0 = (ts * 4 + tt) * 128
                x_t = xpool.tile([128, D], F32, tag="x")
                nc.sync.dma_start(x_t[:], x[t0:t0 + 128, :])
                scr = sqpool.tile([128, D], BF16, tag="scr")
                ssq = spool.tile([128, 1], F32, tag="s")
                nc.scalar.activation(scr[:], x_t[:], AF.Square, accum_out=ssq[:])
                nrm = spool.tile([128, 1], F32, tag="s")
                # sqrt(ssq/D) = ||x||/sqrt(D); clamp matches ref max(||x||,1e-12)
                nc.scalar.activation(nrm[:], ssq[:], AF.Sqrt, scale=1.0 / D)
                nc.vector.tensor_scalar_max(nrm[:], nrm[:], 1e-12 / (D ** 0.5))
                s_t = spool.tile([128, 1], F32, tag="s")
                nc.vector.reciprocal(s_t[:], nrm[:])
                xb = xpool.tile([128, D], BF16, tag="xb")
                nc.gpsimd.tensor_scalar(
                    out=xb[:], in0=x_t[:], scalar1=s_t[:], scalar2=None,
                    op0=ALU.mult,
                )
                xbs.append(xb)

            xt = xtpool.tile([128, DC, 512], BF16, tag="xt")
            for c in range(DC):
                pxt = psWork.tile([128, 512], BF16, tag="wk", name="pxt")
                for tt in range(4):
                    nc.tensor.matmul(
                        pxt[:, tt * 128:(tt + 1) * 128],
                        xbs[tt][:, c * 128:(c + 1) * 128],
                        identb[:],
                        is_transpose=True, start=(tt == 0), stop=(tt == 3),
                    )
                nc.vector.tensor_copy(xt[:, c, :], pxt[:])

            for name in ("q", "k", "v"):
                pp = psWork.tile([128, 512], F32, tag="wk", name="pp")
                wt = w_sb[name]
                for c in range(DC):
                    nc.tensor.matmul(
                        pp[:], wt[:, c, :], xt[:, c, :],
                        start=(c == 0), stop=(c == DC - 1),
                    )
                if name == "q":
                    nc.vector.tensor_copy(qT[:, ts * 512:(ts + 1) * 512], pp[:])
                elif name == "k":
                    nc.vector.tensor_copy(kT[:, ts * 512:(ts + 1) * 512], pp[:])
                else:
                    vt = vtpool.tile([128, 512], BF16, tag="vt")
                    nc.vector.tensor_copy(vt[:], pp[:])
                    for tt in range(4):
                        jj = ts * 4 + tt
                        pv = psWork.tile([128, 128], BF16, tag="wk", name="pv")
                        nc.tensor.matmul(
                            pv[:], vt[:, tt * 128:(tt + 1) * 128],
                            identb[:], is_transpose=True,
                        )
                        dst = v_ext[:, jj * VW:(jj + 1) * VW]
                        dst = dst.rearrange("p (h w) -> p h w", h=2)[:, :, 0:64]
                        src = pv.rearrange("p (h w) -> p h w", h=2)
                        nc.vector.tensor_copy(dst, src)

        def emit_ib(ib):
            """Causal attention for query superblock [ib*512, (ib+1)*512)."""
            njb = 4 * ib + 4
            i0 = ib * 512
            avs = [psAv.tile([65, 512], F32, tag="av", name=f"av{_h}")
                   for _h in range(2)]
            for jb in range(njb):
                m = jb - 4 * ib
                off = 128 * m if m >= 0 else 0
                pss = psSim.tile([128, 1024], F32, tag="sim")
                for head in range(2):
                    hb = head * 64
                    nc.tensor.matmul(
                        pss[:, head * 512 + off:head * 512 + 512],
                        kT[hb:hb + 64, jb * 128:(jb + 1) * 128],
                        qT[hb:hb + 64, i0 + off:i0 + 512],
                    )
                ex = expool.tile([128, 1024], BF16, tag="ex")
                ex3 = ex.rearrange("p (h q) -> p h q", h=2)[:, :, off:512]
                pss3 = pss.rearrange("p (h q) -> p h q", h=2)[:, :, off:512]
                nc.scalar.activation(ex3, pss3, AF.Exp)
                if m >= 0:
                    # zero key p > query f on the 128-wide diagonal strip
                    exd = ex.rearrange("p (h q) -> p h q", h=2)[:, :, off:off + 128]
                    trid = tri.rearrange("p (h q) -> p h q", h=2)
                    nc.vector.tensor_tensor(
                        out=exd, in0=exd, in1=trid, op=ALU.mult)
                exr = ex.rearrange("p (h q) -> p h q", h=2)
                for head in range(2):
                    nc.tensor.matmul(
                        avs[head][:, off:512],
                        v_ext[:, jb * VW + head * 65:jb * VW + head * 65 + 65],
                        exr[:, head, off:512],
                        start=(jb == 0), stop=(jb == njb - 1),
                    )
            for head in range(2):
                o_sb = opool.tile([65, 512], F32, tag="o")
                nc.vector.tensor_copy(o_sb[:], avs[head][:])
                pst = psWork.tile([128, 260], F32, tag="wk", name="pst")
                for k4 in range(4):
                    nc.tensor.matmul(
                        pst[:, k4 * 65:(k4 + 1) * 65],
                        o_sb[:, k4 * 128:(k4 + 1) * 128],
                        ident[0:65, 0:65],
                        is_transpose=True, start=(k4 == 0), stop=(k4 == 3),
                    )
                for k4 in range(4):
                    rd = spool.tile([128, 1], F32, tag="s")
                    nc.vector.reciprocal(rd[:], pst[:, k4 * 65 + 64:k4 * 65 + 65])
                    fin = finpool.tile([128, DH], F32, tag="fin")
                    nc.vector.tensor_scalar(
                        out=fin[:], in0=pst[:, k4 * 65:k4 * 65 + 64],
                        scalar1=rd[:], scalar2=None, op0=ALU.mult,
                    )
                    r0 = i0 + k4 * 128
                    nc.sync.dma_start(out[head, r0:r0 + 128, :], fin[:])

        # interleave so attention(ib) overlaps projections(ts=ib+1)
        emit_ts(0)
        for k in range(1, NSUP):
            emit_ts(k)
            emit_ib(k - 1)
        emit_ib(NSUP - 1)

    nc.compile()
    return nc


_NC = None


def _get_program():
    global _NC
    if _NC is None:
        _NC = build_program()
    return _NC


def make_in_maps(x, gamma, w_qkv):
    x = np.ascontiguousarray(np.asarray(x, dtype=np.float32))
    gamma = np.ascontiguousarray(np.asarray(gamma, dtype=np.float32))
    w_qkv = np.asarray(w_qkv, dtype=np.float32)
    in_maps = []
    for c in range(8):
        b = c // 4
        h0 = 2 * (c % 4)
        in_maps.append({
            "x": x[b],
            "wq": np.ascontiguousarray(w_qkv[:, h0 * 64:(h0 + 2) * 64]),
            "wk": np.ascontiguousarray(w_qkv[:, 512 + h0 * 64:512 + (h0 + 2) * 64]),
            "wv": np.ascontiguousarray(w_qkv[:, 1024 + h0 * 64:1024 + (h0 + 2) * 64]),
            "gamma": gamma,
        })
    return in_maps


def gather_out(results):
    out = np.empty((B, HEADS, N, DH), dtype=np.float32)
    for c in range(8):
        b = c // 4
        h0 = 2 * (c % 4)
        out[b, h0:h0 + 2] = results[c]["out"]
    return out


def kernel(x, gamma, w_qkv, _trace=False):
    from concourse.bass_utils import run_bass_kernel_spmd

    nc = _get_program()
    in_maps = make_in_maps(x, gamma, w_qkv)
    res = run_bass_kernel_spmd(nc, in_maps, list(range(8)), trace=_trace)
    out = gather_out(res.results)
    if _trace:
        return out, res
    return out


# revision 7
# speedup vs baseline: 2.3130x; 2.3130x over previous
"""Causal attention (RMSNorm + QKV proj + causal softmax attention) on 8 TRN2
NeuronCores — bf16 matmul version.

Math (per reference):
    xn   = x / max(||x_row||, 1e-12) * sqrt(D) * gamma
    qkv  = xn @ w_qkv            -> q,k,v heads of dim 64
    q   *= D**-0.5
    out[b,h] = softmax_causal(q k^T) v

Sharding: 16 (batch, head) pairs over 8 cores -> core c gets batch c//4 and
heads {2*(c%4), 2*(c%4)+1}. Each core runs the same single-core Bass program
(SPMD) on its shard; gamma/weight slices are replicated per core.

Per-core kernel layout (all PE matmuls in bf16 — fp32 runs power-throttled
at ~half rate on TRN2; bf16 streams 1 row/cycle):
  - row scale s = sqrt(D)/||x_row||: square+accum on ScalarE, sqrt/recip on
    Scalar/DVE; x scaled to bf16 on GpSimd (otherwise-idle engine)
  - x transposed to d-major via PE transposes in bf16 (PSUM bf16 tiles)
  - projections: W (gamma folded, q also 1/32) bf16 stationary, xT moving ->
    qT/kT [128=2*64 feat, 4096] bf16; vT re-transposed and packed bf16 as
    [v_h0 | ones | v_h1 | ones] so the AV matmul also yields the softmax
    denominator as output row 64.
  - attention transposed: simT[j,i] = kT.T qT per 128-key x 512-query block,
    both heads' blocks land in one 2-bank PSUM tile [128,1024] so a single
    ScalarE exp instruction covers both; causal diagonal strip zeroed by a
    precomputed triangular bf16 mask multiply on DVE; AV accumulates in PSUM
    over key blocks. Final [65,512] blocks PE-transposed back to token-major,
    divided by the denominator row, DMA'd out.
  - ts (projection superblocks) and ib (attention superblocks) are emitted
    interleaved so every engine queue pipelines across the two phases.
"""

import numpy as np
from contextlib import ExitStack

import concourse.bass as bass
import concourse.tile as tile
from concourse import bacc, mybir
from concourse.masks import make_identity

F32 = mybir.dt.float32
BF16 = mybir.dt.bfloat16
AF = mybir.ActivationFunctionType
ALU = mybir.AluOpType

B, N, D = 2, 4096, 1024
HEADS, DH = 8, 64
NT = N // 128      # 32 token tiles
NSUP = N // 512    # 8 token superblocks
DC = D // 128      # 8 contraction chunks
VW = 2 * (DH + 1)  # 130: [v_h0 | ones | v_h1 | ones] per key tile


def build_program():
    nc = bacc.Bacc("TRN2", target_bir_lowering=False, debug=False)

    x = nc.dram_tensor("x", [N, D], F32, kind="ExternalInput").ap()
    wq = nc.dram_tensor("wq", [D, 128], F32, kind="ExternalInput").ap()
    wk = nc.dram_tensor("wk", [D, 128], F32, kind="ExternalInput").ap()
    wv = nc.dram_tensor("wv", [D, 128], F32, kind="ExternalInput").ap()
    gamma = nc.dram_tensor("gamma", [D], F32, kind="ExternalInput").ap()
    out = nc.dram_tensor("out", [2, N, DH], F32, kind="ExternalOutput").ap()

    with tile.TileContext(nc) as tc, ExitStack() as ctx:
        consts = ctx.enter_context(tc.tile_pool(name="consts", bufs=1))
        wpool = ctx.enter_context(tc.tile_pool(name="wpool", bufs=1))
        resid = ctx.enter_context(tc.tile_pool(name="resid", bufs=1))
        xpool = ctx.enter_context(tc.tile_pool(name="xpool", bufs=5))
        sqpool = ctx.enter_context(tc.tile_pool(name="sqpool", bufs=2))
        spool = ctx.enter_context(tc.tile_pool(name="spool", bufs=12))
        xtpool = ctx.enter_context(tc.tile_pool(name="xtpool", bufs=2))
        vtpool = ctx.enter_context(tc.tile_pool(name="vtpool", bufs=2))
        expool = ctx.enter_context(tc.tile_pool(name="expool", bufs=6))
        opool = ctx.enter_context(tc.tile_pool(name="opool", bufs=2))
        finpool = ctx.enter_context(tc.tile_pool(name="finpool", bufs=4))
        psSim = ctx.enter_context(tc.tile_pool(name="psSim", bufs=2, space="PSUM"))
        psAv = ctx.enter_context(tc.tile_pool(name="psAv", bufs=2, space="PSUM"))
        psWork = ctx.enter_context(tc.tile_pool(name="psWork", bufs=2, space="PSUM"))

        # ---- constants ---------------------------------------------------
        ident = consts.tile([128, 128], F32)
        make_identity(nc, ident[:])
        identb = consts.tile([128, 128], BF16)
        nc.vector.tensor_copy(identb[:], ident[:])

        # tri[p, h*128+f] = 1 if p <= f else 0 (causal keep-mask, bf16)
        tri = consts.tile([128, 256], BF16)
        nc.vector.memset(tri[:], 1.0)
        for h in range(2):
            nc.gpsimd.affine_select(
                out=tri[:, h * 128:(h + 1) * 128],
                in_=tri[:, h * 128:(h + 1) * 128],
                compare_op=ALU.is_ge, fill=0.0, base=0,
                channel_multiplier=-1, pattern=[[1, 128]],
            )

        # ---- weights: fold gamma (and D**-0.5 for q), cast bf16 ----------
        w_raw = {}
        w_sb = {}
        for name, w in (("q", wq), ("k", wk), ("v", wv)):
            t = wpool.tile([128, DC, 128], F32, tag=f"wr{name}")
            for c in range(DC):
                nc.sync.dma_start(t[:, c, :], w[c * 128:(c + 1) * 128, :])
            w_raw[name] = t
            w_sb[name] = wpool.tile([128, DC, 128], BF16, tag=f"w{name}", name=f"w_{name}")

        g_sb = wpool.tile([128, DC], F32, tag="g")
        nc.sync.dma_start(g_sb[:], gamma.rearrange("(c p) -> p c", p=128))
        gq_sb = wpool.tile([128, DC], F32, tag="gq")
        nc.scalar.mul(gq_sb[:], g_sb[:], float(D) ** -0.5)

        for name, gt in (("q", gq_sb), ("k", g_sb), ("v", g_sb)):
            for c in range(DC):
                nc.vector.tensor_scalar_mul(
                    w_sb[name][:, c, :], w_raw[name][:, c, :], gt[:, c:c + 1])

        # ---- residents ---------------------------------------------------
        qT = resid.tile([128, N], BF16, tag="qT")
        kT = resid.tile([128, N], BF16, tag="kT")
        # v_ext cols per key tile jj: [v_h0(64) | 1 | v_h1(64) | 1]
        v_ext = resid.tile([128, NT * VW], BF16, tag="vext")
        nc.vector.memset(v_ext[:], 1.0)

        # ---- phase emitters ---------------------------------------------
        def emit_ts(ts):
            """Norm + transpose + projections for tokens [ts*512, (ts+1)*512)."""
            x_ts = []
            ssq4 = spool.tile([128, 4], F32, tag="s4")
            for tt in range(4):
                t0 = (ts * 4 + tt) * 128
                x_t = xpool.tile([128, D], F32, tag="x")
                nc.sync.dma_start(x_t[:], x[t0:t0 + 128, :])
                scr = sqpool.tile([128, D], BF16, tag="scr")
                nc.scalar.activation(
                    scr[:], x_t[:], AF.Square, accum_out=ssq4[:, tt:tt + 1])
                x_ts.append(x_t)
            # s = (max(ssq,1e-24)/D)^-0.5 = sqrt(D)/max(||x||,1e-12), via
            # ln+exp — both live in the same act table set as the attention
            # exp, so the scalar engine never reloads tables.
            nc.vector.tensor_scalar_max(ssq4[:], ssq4[:], 1e-24)
            lssq = spool.tile([128, 4], F32, tag="s4")
            nc.scalar.activation(lssq[:], ssq4[:], AF.Ln, scale=1.0 / D)
            s4 = spool.tile([128, 4], F32, tag="s4")
            nc.scalar.activation(s4[:], lssq[:], AF.Exp, scale=-0.5)

            xbs = []
            for tt in range(4):
                xb = xpool.tile([128, D], BF16, tag="xb")
                nc.vector.tensor_scalar(
                    out=xb[:], in0=x_ts[tt][:], scalar1=s4[:, tt:tt + 1],
                    scalar2=None, op0=ALU.mult,
                )
                xbs.append(xb)

            xt = xtpool.tile([128, DC, 512], BF16, tag="xt")
            for c in range(DC):
                pxt = psWork.tile([128, 512], BF16, tag="wk", name="pxt")
                for tt in range(4):
                    nc.tensor.matmul(
                        pxt[:, tt * 128:(tt + 1) * 128],
                        xbs[tt][:, c * 128:(c + 1) * 128],
                        identb[:],
                        is_transpose=True, start=(tt == 0), stop=(tt == 3),
                    )
                nc.vector.tensor_copy(xt[:, c, :], pxt[:])

            for name in ("q", "k", "v"):
                pp = psWork.tile([128, 512], F32, tag="wk", name="pp")
                wt = w_sb[name]
                for c in range(DC):
                    nc.tensor.matmul(
                        pp[:], wt[:, c, :], xt[:, c, :],
                        start=(c == 0), stop=(c == DC - 1),
                    )
                if name == "q":
                    nc.vector.tensor_copy(qT[:, ts * 512:(ts + 1) * 512], pp[:])
                elif name == "k":
                    nc.vector.tensor_copy(kT[:, ts * 512:(ts + 1) * 512], pp[:])
                else:
                    vt = vtpool.tile([128, 512], BF16, tag="vt")
                    nc.vector.tensor_copy(vt[:], pp[:])
                    for tt in range(4):
                        jj = ts * 4 + tt
                        pv = psWork.tile([128, 128], BF16, tag="wk", name="pv")
                        nc.tensor.matmul(
                            pv[:], vt[:, tt * 128:(tt + 1) * 128],
                            identb[:], is_transpose=True,
                        )
                        dst = v_ext[:, jj * VW:(jj + 1) * VW]
                        dst = dst.rearrange("p (h w) -> p h w", h=2)[:, :, 0:64]
                        src = pv.rearrange("p (h w) -> p h w", h=2)
                        nc.vector.tensor_copy(dst, src)

        def emit_ib(ib):
            """Causal attention for query superblock [ib*512, (ib+1)*512)."""
            njb = 4 * ib + 4
            i0 = ib * 512
            avs = [psAv.tile([65, 512], F32, tag="av", name=f"av{_h}")
                   for _h in range(2)]
            for jb in range(njb):
                m = jb - 4 * ib
                off = 128 * m if m >= 0 else 0
                pss = psSim.tile([128, 1024], F32, tag="sim")
                for head in range(2):
                    hb = head * 64
                    nc.tensor.matmul(
                        pss[:, head * 512 + off:head * 512 + 512],
                        kT[hb:hb + 64, jb * 128:(jb + 1) * 128],
                        qT[hb:hb + 64, i0 + off:i0 + 512],
                    )
                ex = expool.tile([128, 1024], BF16, tag="ex")
                ex3 = ex.rearrange("p (h q) -> p h q", h=2)[:, :, off:512]
                pss3 = pss.rearrange("p (h q) -> p h q", h=2)[:, :, off:512]
                nc.scalar.activation(ex3, pss3, AF.Exp)
                if m >= 0:
                    # zero key p > query f on the 128-wide diagonal strip
                    exd = ex.rearrange("p (h q) -> p h q", h=2)[:, :, off:off + 128]
                    trid = tri.rearrange("p (h q) -> p h q", h=2)
                    nc.vector.tensor_tensor(
                        out=exd, in0=exd, in1=trid, op=ALU.mult)
                exr = ex.rearrange("p (h q) -> p h q", h=2)
                for head in range(2):
                    nc.tensor.matmul(
                        avs[head][:, off:512],
                        v_ext[:, jb * VW + head * 65:jb * VW + head * 65 + 65],
                        exr[:, head, off:512],
                        start=(jb == 0), stop=(jb == njb - 1),
                    )
            for head in range(2):
                o_sb = opool.tile([65, 512], F32, tag="o")
                nc.vector.tensor_copy(o_sb[:], avs[head][:])
                pst = psWork.tile([128, 260], F32, tag="wk", name="pst")
                for k4 in range(4):
                    nc.tensor.matmul(
                        pst[:, k4 * 65:(k4 + 1) * 65],
                        o_sb[:, k4 * 128:(k4 + 1) * 128],
                        ident[0:65, 0:65],
                        is_transpose=True, start=(k4 == 0), stop=(k4 == 3),
                    )
                for k4 in range(4):
                    rd = spool.tile([128, 1], F32, tag="s")
                    nc.vector.reciprocal(rd[:], pst[:, k4 * 65 + 64:k4 * 65 + 65])
                    fin = finpool.tile([128, DH], F32, tag="fin")
                    nc.vector.tensor_scalar(
                        out=fin[:], in0=pst[:, k4 * 65:k4 * 65 + 64],
                        scalar1=rd[:], scalar2=None, op0=ALU.mult,
                    )
                    r0 = i0 + k4 * 128
                    nc.sync.dma_start(out[head, r0:r0 + 128, :], fin[:])

        # interleave so attention(ib) overlaps projections(ts=ib+1)
        emit_ts(0)
        for k in range(1, NSUP):
            emit_ts(k)
            emit_ib(k - 1)
        emit_ib(NSUP - 1)

    nc.compile()
    return nc


_NC = None


def _get_program():
    global _NC
    if _NC is None:
        _NC = build_program()
    return _NC


def make_in_maps(x, gamma, w_qkv):
    x = np.ascontiguousarray(np.asarray(x, dtype=np.float32))
    gamma = np.ascontiguousarray(np.asarray(gamma, dtype=np.float32))
    w_qkv = np.asarray(w_qkv, dtype=np.float32)
    in_maps = []
    for c in range(8):
        b = c // 4
        h0 = 2 * (c % 4)
        in_maps.append({
            "x": x[b],
            "wq": np.ascontiguousarray(w_qkv[:, h0 * 64:(h0 + 2) * 64]),
            "wk": np.ascontiguousarray(w_qkv[:, 512 + h0 * 64:512 + (h0 + 2) * 64]),
            "wv": np.ascontiguousarray(w_qkv[:, 1024 + h0 * 64:1024 + (h0 + 2) * 64]),
            "gamma": gamma,
        })
    return in_maps


def gather_out(results):
    out = np.empty((B, HEADS, N, DH), dtype=np.float32)
    for c in range(8):
        b = c // 4
        h0 = 2 * (c % 4)
        out[b, h0:h0 + 2] = results[c]["out"]
    return out


def kernel(x, gamma, w_qkv, _trace=False):
    from concourse.bass_utils import run_bass_kernel_spmd

    nc = _get_program()
    in_maps = make_in_maps(x, gamma, w_qkv)
    res = run_bass_kernel_spmd(nc, in_maps, list(range(8)), trace=_trace)
    out = gather_out(res.results)
    if _trace:
        return out, res
    return out
